# revision 11
# baseline (speedup 1.0000x reference)
"""AdaptiveTripletLoss on 8 TRN2 NeuronCores (Bass/Tile, SPMD), v5.

loss = mean over rows of relu(1.5*hp - hn + 0.5); hp/hn = hardest
same/other-label distances on L2-normalized embeddings; d^2 = 2-2g.

Per core (1024 rows x 8192 cols of the Gram matrix G'):
  A manually rotated persistent PSUM ring [128,4096] (4 bank-pair slots)
  holds G' tiles; ops run at [128,2048] granularity (two slots) to halve
  per-instruction overhead. Same-label cols sit in a +-96 band around the
  sorted diagonal (slots 0,1 of each row-tile) and get -8 there via a
  one-hot matmul, so a global row-max gives hn and one narrow windowed
  row-min per row-tile gives hp.
  Pair roles:
    A: ScalarE exp(beta*(g-bias)) PSUM->SBUF bf16 ring; one DVE 4x
       tensor_scalar pass per row-tile sums the ring (LSE partial).
    V: one DVE tensor_reduce(max) per pair (exact partial max).
  Partials go to DRAM; the host does exact ln/sqrt/relu/mean in f64.
Host prep: normalize/sort/transpose + one-hot, off-device. PE warm-up
dummies run during the input-DMA wait so HAM reaches 2.4 GHz early.
"""

import sys

sys.path.insert(0, "/opt/trn_rl_repo")

import numpy as np

N_CORES = 8
B, D, NCLS = 8192, 128, 128
BC = B // N_CORES
ROLL = 512
MC = 96
M_TILES = 8
U_COLS = 1024
N_UNITS = B // U_COLS
BETA = 160.0
LSE_BIAS = 0.6
N_WARM = 18

# leading 'A' units per row-tile (even); the rest form V-pairs
_RING_K = {0: 6, 1: 6, 2: 6, 3: 4, 4: 6, 5: 4, 6: 6, 7: 6}

_cache = {}


def _schedule():
    sched = []
    v_k = 0
    for m in range(M_TILES):
        c0 = ROLL + m * 128
        w0, w1 = c0 - MC + 1, c0 + 127 + MC + 1
        k = _RING_K[m]
        vpairs = []
        for u in range(k, N_UNITS, 2):
            vpairs.append((u, v_k))
            v_k += 1
        hh = sorted({w0 // 512, (w1 - 1) // 512})
        sched.append(dict(c0=c0, k=k, vpairs=vpairs, w0=w0, w1=w1, hh=hh))
    return sched, v_k


def _build():
    import concourse.tile as tile
    from concourse import bacc, mybir

    f32 = mybir.dt.float32
    bf16 = mybir.dt.bfloat16
    AX = mybir.AxisListType
    OP = mybir.AluOpType
    AF = mybir.ActivationFunctionType
    from concourse.bass import MemorySpace

    sched, v_k = _schedule()

    nc = bacc.Bacc("TRN2", target_bir_lowering=False, debug=False,
                   num_devices=N_CORES)
    emb_ext = nc.dram_tensor("emb", [128, B], bf16, kind="ExternalInput")
    hw_ext = nc.dram_tensor("hw", [NCLS, 3072], bf16, kind="ExternalInput")
    gmax_ext = nc.dram_tensor("gmax", [128, v_k], f32, kind="ExternalOutput")
    gsum_ext = nc.dram_tensor("gsum", [128, M_TILES], f32,
                              kind="ExternalOutput")
    gmin_ext = nc.dram_tensor("gmin", [128, M_TILES], f32,
                              kind="ExternalOutput")

    with tile.TileContext(nc) as tc:
        with (
            tc.tile_pool(name="persist", bufs=1) as pp,
            tc.tile_pool(name="rng", bufs=2) as rng_pool,
            tc.tile_pool(name="PSP", bufs=1, space=MemorySpace.PSUM) as PSP,
        ):
            et = pp.tile([128, B], bf16, name="et_sb", tag="et_sb")
            hw = pp.tile([NCLS, 3072], bf16, name="hw_sb", tag="hw_sb")
            gmaxs = pp.tile([128, v_k], f32, name="gmaxs", tag="gmaxs")
            gsums = pp.tile([128, M_TILES], f32, name="gsums", tag="gsums")
            gmins = pp.tile([128, M_TILES], f32, name="gmins", tag="gmins")
            bexp = pp.tile([128, 1], f32, name="bexp", tag="bexp")
            warm = pp.tile([128, 1], f32, name="warm", tag="warm")
            wsc = pp.tile([128, 128], bf16, name="wsc", tag="wsc")
            tscr = pp.tile([128, 6144], bf16, name="tscr", tag="tscr")
            PS = PSP.tile([128, 4096], f32, name="ps_ring", tag="ps_ring")

            emb_ap = emb_ext.ap()
            nc.sync.dma_start(et[:, 0:1024], emb_ap[:, 0:1024])
            nc.sync.dma_start(et[:, 1024:2048], emb_ap[:, 1024:2048])
            nc.sync.dma_start(hw[:], hw_ext.ap())
            nc.sync.dma_start(et[:, 2048:B], emb_ap[:, 2048:B])
            htw = hw[:, 0:2048]
            hnw = hw[:, 2048:3072]

            nc.gpsimd.memset(wsc[:], 0.03125)
            nc.gpsimd.memset(bexp[:], -float(BETA * LSE_BIAS))
            nc.gpsimd.memset(warm[:], 0.0)
            # exp table load off the critical path
            nc.scalar.activation(warm[:], warm[:], AF.Exp,
                                 bias=bexp[:], scale=0.0)
            # PE warm-up during the input-DMA wait
            for _ in range(N_WARM):
                nc.tensor.matmul(PS[:, 0:128], wsc[:], wsc[:],
                                 start=True, stop=True)

            for m in range(M_TILES):
                ent = sched[m]
                c0 = ent["c0"]
                k = ent["k"]
                stat = et[:, c0:c0 + 128]
                ring = rng_pool.tile([128, k * U_COLS], bf16, tag=f"ring{k}")
                for u in range(0, N_UNITS, 2):
                    lo = ((m * N_UNITS + u) % 4) * U_COLS
                    for du in range(2):
                        uu = u + du
                        for cc in range(2):
                            a = du * U_COLS + cc * 512
                            winpart = u == 0 and (2 * uu + cc) in ent["hh"]
                            nc.tensor.matmul(
                                PS[:, lo + a:lo + a + 512], stat,
                                et[:, uu * U_COLS + cc * 512:
                                   uu * U_COLS + cc * 512 + 512],
                                start=True, stop=not winpart)
                    if u == 0:
                        hstat = hnw[:, c0 - 512:c0 - 512 + 128]
                        for ch in ent["hh"]:
                            nc.tensor.matmul(
                                PS[:, lo + ch * 512:lo + (ch + 1) * 512],
                                hstat, htw[:, ch * 512:(ch + 1) * 512],
                                start=False, stop=True)
                        # one narrow windowed min for the whole row-tile
                        nc.vector.tensor_reduce(
                            gmins[:, m:m + 1],
                            PS[:, lo + ent["w0"]:lo + ent["w1"]],
                            axis=AX.X, op=OP.min)
                    if u < k:
                        nc.scalar.activation(
                            ring[:, u * U_COLS:(u + 2) * U_COLS],
                            PS[:, lo:lo + 2048],
                            AF.Exp, bias=bexp[:], scale=float(BETA))
                    else:
                        col = next(c_ for (ua, c_) in ent["vpairs"]
                                   if ua == u)
                        nc.vector.tensor_reduce(
                            gmaxs[:, col:col + 1], PS[:, lo:lo + 2048],
                            axis=AX.X, op=OP.max)
                # ring LSE sum at 4x (bf16 SBUF tensor_scalar with accum)
                nc.vector.tensor_scalar(
                    tscr[:, 0:k * U_COLS], ring[:], 1.0, 0.0,
                    op0=OP.mult, op1=OP.add,
                    accum_out=gsums[:, m:m + 1])

            nc.sync.dma_start(gmax_ext.ap(), gmaxs[:])
            nc.sync.dma_start(gsum_ext.ap(), gsums[:])
            nc.sync.dma_start(gmin_ext.ap(), gmins[:])

    nc.compile()
    return nc


def _get_nc():
    if "nc" not in _cache:
        _cache["nc"] = _build()
    return _cache["nc"]


def _prep_inputs(embeddings, labels):
    import ml_dtypes

    emb = np.ascontiguousarray(np.asarray(embeddings, dtype=np.float32))
    lab = np.asarray(labels).astype(np.int64).ravel()
    assert emb.shape == (B, D) and lab.shape == (B,)

    counts = np.bincount(lab, minlength=NCLS)
    present = counts[counts > 0]
    assert present.max() <= MC, f"class too large for window: {present.max()}"
    assert present.min() >= 2, "singleton class unsupported"

    norm = np.maximum(np.linalg.norm(emb, axis=1, keepdims=True), 1e-12)
    emb_n = emb / norm
    perm = np.argsort(lab, kind="stable")
    emb_s = emb_n[perm]
    lab_s = lab[perm]

    in_maps = []
    for c in range(N_CORES):
        shift = ROLL - BC * c
        emb_l = np.roll(emb_s, shift, axis=0)
        lab_l = np.roll(lab_s, shift)
        htw = (lab_l[None, :2048] == np.arange(NCLS)[:, None]).astype(
            ml_dtypes.bfloat16)
        hnw = (-8.0 * htw[:, 512:1536]).astype(ml_dtypes.bfloat16)
        et = np.ascontiguousarray(emb_l.T.astype(ml_dtypes.bfloat16))
        in_maps.append({
            "emb": et,
            "hw": np.ascontiguousarray(np.concatenate([htw, hnw], axis=1)),
        })
    return in_maps


def kernel(embeddings, labels, _trace=False):
    from concourse.bass_utils import run_bass_kernel_spmd

    nc = _get_nc()
    in_maps = _prep_inputs(embeddings, labels)
    res = run_bass_kernel_spmd(nc, in_maps, core_ids=list(range(N_CORES)),
                               trace=_trace)
    if _trace:
        _cache["last_exec_time_ns"] = res.exec_time_ns
        _cache["last_results"] = res

    sched, v_k = _schedule()
    total = 0.0
    for c in range(N_CORES):
        r = res.results[c]
        gmax = np.asarray(r["gmax"], dtype=np.float64)
        gsum = np.asarray(r["gsum"], dtype=np.float64)
        gmin = np.asarray(r["gmin"], dtype=np.float64)
        lse = LSE_BIAS + np.log(np.maximum(gsum, 1e-300)) / BETA
        for m in range(M_TILES):
            ent = sched[m]
            vcols = [c_ for (_, c_) in ent["vpairs"]]
            hn_g = np.maximum(gmax[:, vcols].max(axis=1), lse[:, m])
            hp_g = gmin[:, m]
            hn = np.sqrt(np.maximum(2.0 - 2.0 * hn_g, 0.0))
            hp = np.sqrt(np.maximum(-14.0 - 2.0 * hp_g, 0.0))
            total += np.maximum(1.5 * hp - hn + 0.5, 0.0).sum()
    return np.float32(total / B)
